# revision 17
# baseline (speedup 1.0000x reference)
"""DirectedLowRankEdgeScorer TRN2 Bass kernel (8 NeuronCores, SPMD) — v4.

logits[b,l,e] = sum_r a[b,I[e],r] * gamma[l,r] * b[b,J[e],r]
  a = relu(H@W1s+b1s)@W2s+b2s,  b = relu(H@W1d+b1d)@W2d+b2d,  H = X[:,-1]

Per-core plan (core c owns node I-shard [c*NP, (c+1)*NP)):
  1. MLP over the shard -> 256B records rec[n]=[a0 a1 b0 b1] (fp32) in DRAM;
     AllGather halves -> rec_h1/rec_h2; local a-vals also kept in SBUF as
     atab [128, 49, 32] bf16 (node-major windows of 128).
  2. Edges are assigned to cores by I-shard (avg out-degree 32). Per 128-node
     window w: 4096 slots = 2048 per J-half. A-side values come from a one-hot
     expansion matmul aI = atab[:,w,:]^T @ M (M built on-device by DVE
     is_equal(posrep, iota)); NO A-side gather descriptors. B-side: one
     2048-idx dma_gather per (window, J-half).
  3. Per bank (2048 slots): DVE packs b-vals, PE transposes to val-major,
     prod = aI * bT (bf16), block-diag gamma matmul -> [96, 512] -> bf16 OUT.
  4. Window-half overflow edges go to 2 spill tile-groups (per J-half),
     processed v2-style with both sides gathered (A from local rec_shard).
"""

import sys
import types

import numpy as np
import ml_dtypes

import bass_rust
import concourse.bass as bass
import concourse.bacc as bacc
import concourse.mybir as mybir
from concourse.bass_utils import run_bass_kernel_spmd
from concourse.tile import TileContext
from concourse.vector_clock import ScopedClock
from concourse.masks import make_identity
from concourse.tile import add_dep_helper

BF16 = ml_dtypes.bfloat16

B, T, N, C = 2, 8, 50000, 64
HID, R, L, E = 128, 16, 12, 1600000
NCORES = 8
NP = 6272                     # nodes per core shard (49*128)
NPAD = NP * NCORES            # 50176
H1N, H2N = 3200, 3072         # J-half split (per-rank rows in rec_h1/rec_h2)
NWIN = NP // 128              # 49 windows per core
WQ = 2048                     # window slots per J-half (= idxs per B-gather)
SPILL = 4096                  # spill slots per J-half (one v2-style TG each)
NBANK = 2 * NWIN + 4          # 102 banks of 2048 slots
EPAD = NBANK * 2048           # 208896 slots per core


# ---------------------------------------------------------------- patches
def _patched_drain_and_barrier(self, tick_clock, wait_clock):
    nc = self.nc
    probe = nc.sync.drain()
    wait_clock.add_sem_waits(probe.ins, ScopedClock({None: tick_clock.global_clock}))
    si = probe.ins.sync_info
    waits = list(si.on_wait) if si is not None else []
    if len(waits) > 2:
        si.on_wait.clear()
        si.on_wait.extend(waits[:2])
        for k in range(2, len(waits), 2):
            ni = nc.sync.drain().ins
            ni.sync_info = bass_rust.SyncInfo(on_wait=waits[k:k + 2], on_update=[])
    nc.all_engine_barrier()
    assert self.sems is not None
    popped = nc._tile_sem_poison_stack.pop()
    assert popped is self._sem_poison
    nc.clear_and_free_semaphores(list(self.sems.allocated().values()))
    nc.all_engine_barrier()


TileContext._drain_and_barrier = _patched_drain_and_barrier

if "antenv.axon_hooks" not in sys.modules:
    _mod = types.ModuleType("antenv.axon_hooks")
    _state = {"hook": None}
    _mod.set_axon_ntff_profile_hook = lambda h: _state.__setitem__("hook", h)
    _mod.get_axon_ntff_profile_hook = lambda: _state["hook"]
    sys.modules["antenv.axon_hooks"] = _mod
    try:
        import antenv

        antenv.axon_hooks = _mod
    except Exception:
        pass
    try:
        from trn_agent_boot.trn_boot import _ntff_profile_via_ctypes

        _hook = _ntff_profile_via_ctypes("/opt/axon/libaxon_pjrt.so")
        if _hook is not None:
            _mod.set_axon_ntff_profile_hook(_hook)
    except Exception:
        pass


# ---------------------------------------------------------------- device
_PROGRAM_CACHE = {}


def build_program():
    f32, bf16, i16 = mybir.dt.float32, mybir.dt.bfloat16, mybir.dt.int16

    nc = bacc.Bacc("TRN2", target_bir_lowering=False, num_swdge_queues=4)

    HT = nc.declare_dram_parameter("HT", [B, C, NP], f32, isOutput=False)
    W1 = nc.declare_dram_parameter("W1", [2, C, HID], f32, isOutput=False)
    B1 = nc.declare_dram_parameter("B1", [2, HID, 1], f32, isOutput=False)
    W2 = nc.declare_dram_parameter("W2", [2, HID, R], f32, isOutput=False)
    B2 = nc.declare_dram_parameter("B2", [2, 128, R], f32, isOutput=False)
    GBD = nc.declare_dram_parameter("GBD", [128, 96], bf16, isOutput=False)
    IOTA = nc.declare_dram_parameter("IOTA", [128, 1], f32, isOutput=False)
    IDXB = nc.declare_dram_parameter("IDXB", [2 * NWIN, 128, WQ // 16], i16,
                                     isOutput=False)
    SIDXA = nc.declare_dram_parameter("SIDXA", [2, 128, SPILL // 16], i16,
                                      isOutput=False)
    SIDXB = nc.declare_dram_parameter("SIDXB", [2, 128, SPILL // 16], i16,
                                      isOutput=False)
    POSREP = nc.declare_dram_parameter("POSREP", [NWIN, 128, 4096], bf16,
                                       isOutput=False)
    OUT = nc.declare_dram_parameter("OUT", [96, NBANK * 512], bf16, isOutput=True)

    rec_shard = nc.dram_tensor("rec_shard", [NP, 64], f32)
    rec_h1 = nc.dram_tensor("rec_h1", [NCORES * H1N, 64], f32, addr_space="Shared")
    rec_h2 = nc.dram_tensor("rec_h2", [NCORES * H2N, 64], f32, addr_space="Shared")

    with TileContext(nc) as tc:
        with (
            tc.tile_pool(name="const", bufs=1) as constp,
            tc.tile_pool(name="htp", bufs=2) as htp,
            tc.tile_pool(name="h1p", bufs=1) as h1p,
            tc.tile_pool(name="recp", bufs=3) as recp,
            tc.tile_pool(name="posp", bufs=2) as posp,
            tc.tile_pool(name="Mp", bufs=2) as Mp,
            tc.tile_pool(name="gBp", bufs=2) as gBp,
            tc.tile_pool(name="bPkp", bufs=3) as bPkp,
            tc.tile_pool(name="cBp", bufs=3) as cBp,
            tc.tile_pool(name="prodp", bufs=3) as prodp,
            tc.tile_pool(name="outp", bufs=3) as outp,
            tc.tile_pool(name="psT", bufs=2, space="PSUM") as psT,
            tc.tile_pool(name="ps2", bufs=2, space="PSUM") as ps2,
            tc.tile_pool(name="psA", bufs=2, space="PSUM") as psA,
            tc.tile_pool(name="psL", bufs=2, space="PSUM") as psL,
        ):
            w1_s = constp.tile([C, 2, HID], f32)
            nc.sync.dma_start(w1_s[:], W1[:].rearrange("t c h -> c t h"))
            b1_s = constp.tile([HID, 2, 1], f32)
            nc.sync.dma_start(b1_s[:], B1[:].rearrange("t h o -> h t o"))
            w2_s = constp.tile([HID, 2, R], bf16)
            nc.gpsimd.dma_start(w2_s[:], W2[:].rearrange("t h r -> h t r"))
            b2_s = constp.tile([128, 2, R], f32)
            nc.sync.dma_start(b2_s[:], B2[:].rearrange("t p r -> p t r"))
            gbd_s = constp.tile([128, 96], bf16)
            nc.sync.dma_start(gbd_s[:], GBD[:])
            iota_s = constp.tile([128, 1], f32)
            nc.sync.dma_start(iota_s[:], IOTA[:])
            idxb_all = constp.tile([128, 2 * NWIN, WQ // 16], i16)
            nc.sync.dma_start(idxb_all[:], IDXB[:].rearrange("t p x -> p t x"))
            sidxa_s = constp.tile([128, 2, SPILL // 16], i16)
            nc.sync.dma_start(sidxa_s[:], SIDXA[:].rearrange("t p x -> p t x"))
            sidxb_s = constp.tile([128, 2, SPILL // 16], i16)
            nc.sync.dma_start(sidxb_s[:], SIDXB[:].rearrange("t p x -> p t x"))
            ident = constp.tile([128, 128], f32)
            make_identity(nc, ident[:])
            atab = constp.tile([128, NWIN, 32], bf16)

            # ---- MLP passes; each pass ends with its half-shard AllGather
            cc_insts = []
            rec_dmas = []
            for (p0, psz) in ((0, H1N), (H1N, H2N)):
                h1t = {}
                for t in range(2):
                    for b in range(B):
                        h1x = h1p.tile([HID, max(H1N, H2N)], bf16, tag=f"h1_{t}_{b}")
                        h1t[(t, b)] = h1x
                for n0 in range(0, psz, 512):
                    csz = min(512, psz - n0)
                    htc = htp.tile([C, B, 512], f32, tag="ht")
                    nc.sync.dma_start(
                        htc[:, :, :csz],
                        HT[:, :, p0 + n0:p0 + n0 + csz].rearrange("b c n -> c b n"),
                    )
                    for t in range(2):
                        for b in range(B):
                            p1 = psT.tile([HID, 512], f32, tag="px")
                            nc.tensor.matmul(
                                p1[:, :csz],
                                w1_s[:, t, :],
                                htc[:, b, :csz],
                            )
                            nc.scalar.activation(
                                h1t[(t, b)][:, n0:n0 + csz], p1[:, :csz],
                                mybir.ActivationFunctionType.Relu,
                                bias=b1_s[:, t, :], scale=1.0,
                            )
                for s in range(psz // 128):
                    rec = recp.tile([128, 64], f32, tag="rec")
                    for t in range(2):
                        for b in range(B):
                            p2 = ps2.tile([128, R], f32, tag="p2")
                            nc.tensor.matmul(
                                p2[:],
                                h1t[(t, b)][:, s * 128:(s + 1) * 128],
                                w2_s[:, t, :],
                            )
                            co = 32 * t + 16 * b
                            nc.vector.tensor_add(
                                rec[:, co:co + 16], p2[:], b2_s[:, t, :]
                            )
                    n0 = p0 + s * 128
                    nc.vector.tensor_copy(atab[:, n0 // 128, :], rec[:, 0:32])
                    di = nc.sync.dma_start(rec_shard[n0:n0 + 128, :], rec[:])
                    rec_dmas.append(di)
                dst = rec_h1 if p0 == 0 else rec_h2
                cc = nc.gpsimd.collective_compute(
                    "AllGather",
                    mybir.AluOpType.bypass,
                    replica_groups=[list(range(NCORES))],
                    ins=[rec_shard[p0:p0 + psz, :]],
                    outs=[dst[:]],
                )
                for di in rec_dmas:
                    add_dep_helper(cc.ins, di.ins, True, "cc waits rec dmas")
                if cc_insts:
                    add_dep_helper(cc.ins, cc_insts[-1].ins, True, "cc order")
                cc_insts.append(cc)

            # ---- window phase
            qn = 0
            for w in range(NWIN):
                posr = posp.tile([128, 4096], bf16, tag="pos")
                nc.sync.dma_start(posr[:], POSREP[w])
                Mt = Mp.tile([128, 4096], bf16, tag="M")
                nc.vector.tensor_scalar(
                    Mt[:], posr[:], iota_s[:], None, mybir.AluOpType.is_equal
                )
                for h in range(2):
                    rec_src = rec_h1 if h == 0 else rec_h2
                    gB = gBp.tile([128, WQ // 128, 64], f32, tag="gB")
                    gb_i = nc.gpsimd.dma_gather(
                        gB[:], rec_src[:], idxb_all[:, 2 * w + h, :],
                        num_idxs=WQ, num_idxs_reg=WQ, elem_size=64,
                        single_packet=False, queue_num=qn % 4,
                    )
                    qn += 1
                    add_dep_helper(gb_i.ins, cc_insts[h].ins, True, "gather waits cc")

                    bPk = bPkp.tile([128, WQ // 128, 32], f32, tag="bPk")
                    nc.vector.tensor_copy(bPk[:], gB[:, :, 32:64])

                    # expansion: aI bank [128, 512], 4 sub-matmuls
                    aIb = psA.tile([128, 512], f32, tag="aI")
                    for si in range(4):
                        nc.tensor.matmul(
                            aIb[32 * si:32 * (si + 1), :],
                            atab[:, w, :],
                            Mt[:, 2048 * h + 512 * si:2048 * h + 512 * (si + 1)],
                            start=True, stop=True,
                            tile_position=(0, 32 * si),
                        )

                    pTB = psT.tile([128, 512], f32, tag="px")
                    for gg in range(4):
                        nc.tensor.transpose(
                            pTB[:, 128 * gg:128 * (gg + 1)],
                            bPk[:, 4 * gg:4 * (gg + 1), :],
                            ident[:],
                        )
                    cB = cBp.tile([128, 512], f32, tag="cBf")
                    nc.scalar.copy(cB[:], pTB[:])

                    prod = prodp.tile([128, 512], bf16, tag="prod")
                    nc.vector.tensor_mul(prod[:], aIb[:], cB[:])

                    pL = psL.tile([96, 512], f32, tag="pL")
                    nc.tensor.matmul(pL[:], gbd_s[:], prod[:], start=True, stop=True)
                    outS = outp.tile([96, 512], bf16, tag="outS")
                    nc.scalar.copy(outS[:], pL[:])
                    bank = 2 * w + h
                    nc.sync.dma_start(OUT[:, 512 * bank:512 * (bank + 1)], outS[:])

            # ---- spill phase: one v2-style TG per J-half
            for h in range(2):
                rec_src = rec_h1 if h == 0 else rec_h2
                gA = gBp.tile([128, SPILL // 128, 64], f32, tag="sgA")
                ga_i = nc.gpsimd.dma_gather(
                    gA[:], rec_shard[:], sidxa_s[:, h, :],
                    num_idxs=SPILL, num_idxs_reg=SPILL, elem_size=64,
                    single_packet=False, queue_num=qn % 4,
                )
                qn += 1
                for di in rec_dmas:
                    add_dep_helper(ga_i.ins, di.ins, True, "spillA waits rec")
                gB2 = gBp.tile([128, SPILL // 128, 64], f32, tag="sgB")
                gb_i = nc.gpsimd.dma_gather(
                    gB2[:], rec_src[:], sidxb_s[:, h, :],
                    num_idxs=SPILL, num_idxs_reg=SPILL, elem_size=64,
                    single_packet=False, queue_num=qn % 4,
                )
                qn += 1
                add_dep_helper(gb_i.ins, cc_insts[h].ins, True, "spillB waits cc")

                prodS = prodp.tile([128, SPILL // 128, 32], f32, tag="sprod")
                nc.vector.tensor_mul(prodS[:], gA[:, :, 0:32], gB2[:, :, 32:64])
                for jj in range(2):
                    pT = psT.tile([128, 512], f32, tag="px")
                    for gg in range(4):
                        j = 4 * jj + gg
                        nc.tensor.transpose(
                            pT[:, 128 * gg:128 * (gg + 1)],
                            prodS[:, 4 * j:4 * (j + 1), :],
                            ident[:],
                        )
                    cS = cBp.tile([128, 512], bf16, tag="cB")
                    nc.scalar.copy(cS[:], pT[:])
                    pL = psL.tile([96, 512], f32, tag="pL")
                    nc.tensor.matmul(pL[:], gbd_s[:], cS[:], start=True, stop=True)
                    outS = outp.tile([96, 512], bf16, tag="outS")
                    nc.scalar.copy(outS[:], pL[:])
                    bank = 2 * NWIN + 2 * h + jj
                    nc.sync.dma_start(OUT[:, 512 * bank:512 * (bank + 1)], outS[:])

    nc.finalize()
    return nc


# ---------------------------------------------------------------- host
def _wrap_idx(flat_idx, kg):
    """[kg] int16 -> [128, kg//16] wrapped-16, replicated x8."""
    w = flat_idx.reshape(kg // 16, 16).T
    return np.tile(w, (8, 1))


def kernel(X, edge_index, W1s, b1s, W2s, b2s, W1d, b1d, W2d, b2d, gamma):
    X = np.asarray(X)
    edge_index = np.asarray(edge_index)
    H = np.ascontiguousarray(X[:, -1]).astype(np.float32)          # (B, N, C)
    Hp = np.zeros((B, NPAD, C), np.float32)
    Hp[:, :N] = H

    I = edge_index[0].astype(np.int64)
    J = edge_index[1].astype(np.int64)

    # J-side rows in the AllGather'd half tables (per-rank interleaved)
    rJ = J // NP
    iJ = J % NP
    in1 = iJ < H1N
    rowJ = np.where(in1, H1N * rJ + iJ, H2N * rJ + (iJ - H1N))
    hJ = np.where(in1, 0, 1)

    coreof = I // NP
    Iloc = I - coreof * NP
    wof = Iloc // 128
    posof = Iloc % 128

    if () not in _PROGRAM_CACHE:
        _PROGRAM_CACHE[()] = build_program()
    nc = _PROGRAM_CACHE[()]

    W1 = np.stack([W1s, W1d]).astype(np.float32)
    B1 = np.stack([b1s, b1d]).astype(np.float32)[:, :, None]
    W2 = np.stack([W2s, W2d]).astype(np.float32)
    B2 = np.stack(
        [np.tile(b2s[None, :], (128, 1)), np.tile(b2d[None, :], (128, 1))]
    ).astype(np.float32)

    gbd = np.zeros((128, 96), np.float32)
    gT = np.asarray(gamma, np.float32).T
    for g in range(4):
        for b in range(B):
            gbd[32 * g + 16 * b:32 * g + 16 * b + 16,
                24 * g + 12 * b:24 * g + 12 * b + 12] = gT
    GBD = gbd.astype(BF16)
    IOTA = np.arange(128, dtype=np.float32)[:, None]

    # record r -> posrep col within a half's 2048-col block
    r2 = np.arange(WQ)
    colmap = 512 * ((r2 // 128) % 4) + 128 * (r2 // 512) + r2 % 128

    in_maps = []
    unperm = []
    for c in range(NCORES):
        sel = np.nonzero(coreof == c)[0]
        wc, hc = wof[sel], hJ[sel]
        key = 2 * wc + hc
        order = np.argsort(key, kind="stable")
        sel_s = sel[order]
        key_s = key[order]
        cnts = np.bincount(key_s, minlength=2 * NWIN)
        starts = np.zeros(2 * NWIN + 1, np.int64)
        starts[1:] = np.cumsum(cnts)

        posrep = np.full((NWIN, 4096), 255.0, np.float32)
        idxB = np.zeros((2 * NWIN, WQ), np.int16)
        pad_pos = np.full(EPAD, -1, np.int64)
        spills = [[], []]
        for w in range(NWIN):
            for h in range(2):
                k = 2 * w + h
                ed = sel_s[starts[k]:starts[k + 1]]
                if len(ed) > WQ:
                    spills[h].append(ed[WQ:])
                    ed = ed[:WQ]
                ncnt = len(ed)
                posrep[w, 2048 * h + colmap[:ncnt]] = posof[ed]
                idxB[k, :ncnt] = rowJ[ed].astype(np.int16)
                pad_pos[2048 * k:2048 * k + ncnt] = ed

        sidxA = np.zeros((2, SPILL), np.int16)
        sidxB = np.zeros((2, SPILL), np.int16)
        for h in range(2):
            sl = (np.concatenate(spills[h]) if spills[h]
                  else np.empty(0, np.int64))
            assert len(sl) <= SPILL, f"core {c} half {h} spill {len(sl)}"
            scnt = len(sl)
            sidxA[h, :scnt] = Iloc[sl].astype(np.int16)
            sidxB[h, :scnt] = rowJ[sl].astype(np.int16)
            base = 2048 * (2 * NWIN + 2 * h)
            pad_pos[base:base + scnt] = sl
        unperm.append(pad_pos)

        IDXB_w = np.zeros((2 * NWIN, 128, WQ // 16), np.int16)
        for k in range(2 * NWIN):
            IDXB_w[k] = _wrap_idx(idxB[k], WQ)
        SIDXA_w = np.stack([_wrap_idx(sidxA[h], SPILL) for h in range(2)])
        SIDXB_w = np.stack([_wrap_idx(sidxB[h], SPILL) for h in range(2)])
        POSREP_w = np.broadcast_to(
            posrep.astype(BF16)[:, None, :], (NWIN, 128, 4096)
        ).copy()

        HTs = np.ascontiguousarray(
            Hp[:, c * NP:(c + 1) * NP, :].transpose(0, 2, 1)
        )
        in_maps.append({
            "HT": HTs, "W1": W1, "B1": B1, "W2": W2, "B2": B2,
            "GBD": GBD, "IOTA": IOTA, "IDXB": IDXB_w,
            "SIDXA": SIDXA_w, "SIDXB": SIDXB_w, "POSREP": POSREP_w,
        })

    import os
    import tempfile
    trace = bool(os.environ.get("BASS_KERNEL_TRACE"))
    tdir = None
    if trace:
        base = "/root/problem/work"
        tdir = tempfile.mkdtemp(prefix="ktrace_", dir=base if os.path.isdir(base) else None)
    res = run_bass_kernel_spmd(
        nc, in_maps, list(range(NCORES)), trace=trace, tmpdir=tdir,
    )
    if trace:
        kernel.last_trace_dir = tdir
        kernel.last_exec_time_ns = res.exec_time_ns

    logits = np.empty((B, L, E), np.float32)
    for c in range(NCORES):
        dev = res.results[c]["OUT"].astype(np.float32)             # (96, NBANK*512)
        # partition p = 24*k + 12*b + l ; col = 512*Tb + 128*g + e ;
        # padded pos = 2048*Tb + 512*g + 128*k + e
        dv = dev.reshape(4, 2, L, NBANK, 4, 128)                   # (k, b, l, Tb, g, e)
        dv = dv.transpose(1, 2, 3, 4, 0, 5).reshape(B, L, EPAD)
        pad_pos = unperm[c]
        valid = pad_pos >= 0
        logits[:, :, pad_pos[valid]] = dv[:, :, valid]
    return logits


# revision 26
# speedup vs baseline: 1.1338x; 1.1338x over previous
"""DirectedLowRankEdgeScorer TRN2 Bass kernel (8 NeuronCores, SPMD) — v4.

logits[b,l,e] = sum_r a[b,I[e],r] * gamma[l,r] * b[b,J[e],r]
  a = relu(H@W1s+b1s)@W2s+b2s,  b = relu(H@W1d+b1d)@W2d+b2d,  H = X[:,-1]

Per-core plan (core c owns node I-shard [c*NP, (c+1)*NP)):
  1. MLP over the shard -> 256B records rec[n]=[a0 a1 b0 b1] (fp32) in DRAM;
     AllGather halves -> rec_h1/rec_h2; local a-vals also kept in SBUF as
     atab [128, 49, 32] bf16 (node-major windows of 128).
  2. Edges are assigned to cores by I-shard (avg out-degree 32). Per 128-node
     window w: 4096 slots = 2048 per J-half. A-side values come from a one-hot
     expansion matmul aI = atab[:,w,:]^T @ M (M built on-device by DVE
     is_equal(posrep, iota)); NO A-side gather descriptors. B-side: one
     2048-idx dma_gather per (window, J-half).
  3. Per bank (2048 slots): DVE packs b-vals, PE transposes to val-major,
     prod = aI * bT (bf16), block-diag gamma matmul -> [96, 512] -> bf16 OUT.
  4. Window-half overflow edges go to 2 spill tile-groups (per J-half),
     processed v2-style with both sides gathered (A from local rec_shard).
"""

import sys
import types

import numpy as np
import ml_dtypes

import bass_rust
import concourse.bass as bass
import concourse.bacc as bacc
import concourse.mybir as mybir
from concourse.bass_utils import run_bass_kernel_spmd
from concourse.tile import TileContext
from concourse.vector_clock import ScopedClock
from concourse.masks import make_identity
from concourse.tile import add_dep_helper

BF16 = ml_dtypes.bfloat16

B, T, N, C = 2, 8, 50000, 64
HID, R, L, E = 128, 16, 12, 1600000
NCORES = 8
NP = 6272                     # nodes per core shard (49*128)
NPAD = NP * NCORES            # 50176
H1N, H2N = 3200, 3072         # J-half split (per-rank rows in rec_h1/rec_h2)
NWIN = NP // 128              # 49 windows per core
WQ = 2048                     # window slots per J-half (= idxs per B-gather)
SPILL = 4096                  # spill slots per J-half (one v2-style TG each)
NBANK = 2 * NWIN + 4          # 102 banks of 2048 slots
EPAD = NBANK * 2048           # 208896 slots per core


# ---------------------------------------------------------------- patches
def _patched_drain_and_barrier(self, tick_clock, wait_clock):
    nc = self.nc
    probe = nc.sync.drain()
    wait_clock.add_sem_waits(probe.ins, ScopedClock({None: tick_clock.global_clock}))
    si = probe.ins.sync_info
    waits = list(si.on_wait) if si is not None else []
    if len(waits) > 2:
        si.on_wait.clear()
        si.on_wait.extend(waits[:2])
        for k in range(2, len(waits), 2):
            ni = nc.sync.drain().ins
            ni.sync_info = bass_rust.SyncInfo(on_wait=waits[k:k + 2], on_update=[])
    nc.all_engine_barrier()
    assert self.sems is not None
    popped = nc._tile_sem_poison_stack.pop()
    assert popped is self._sem_poison
    nc.clear_and_free_semaphores(list(self.sems.allocated().values()))
    nc.all_engine_barrier()


TileContext._drain_and_barrier = _patched_drain_and_barrier

if "antenv.axon_hooks" not in sys.modules:
    _mod = types.ModuleType("antenv.axon_hooks")
    _state = {"hook": None}
    _mod.set_axon_ntff_profile_hook = lambda h: _state.__setitem__("hook", h)
    _mod.get_axon_ntff_profile_hook = lambda: _state["hook"]
    sys.modules["antenv.axon_hooks"] = _mod
    try:
        import antenv

        antenv.axon_hooks = _mod
    except Exception:
        pass
    try:
        from trn_agent_boot.trn_boot import _ntff_profile_via_ctypes

        _hook = _ntff_profile_via_ctypes("/opt/axon/libaxon_pjrt.so")
        if _hook is not None:
            _mod.set_axon_ntff_profile_hook(_hook)
    except Exception:
        pass


# ---------------------------------------------------------------- device
_PROGRAM_CACHE = {}


def build_program():
    f32, bf16, i16 = mybir.dt.float32, mybir.dt.bfloat16, mybir.dt.int16

    nc = bacc.Bacc("TRN2", target_bir_lowering=False, num_swdge_queues=4)

    HT = nc.declare_dram_parameter("HT", [B, C, NP], f32, isOutput=False)
    W1 = nc.declare_dram_parameter("W1", [2, C, HID], f32, isOutput=False)
    B1 = nc.declare_dram_parameter("B1", [2, HID, 1], f32, isOutput=False)
    W2 = nc.declare_dram_parameter("W2", [2, HID, R], f32, isOutput=False)
    B2 = nc.declare_dram_parameter("B2", [2, 128, R], f32, isOutput=False)
    GBD = nc.declare_dram_parameter("GBD", [128, 96], bf16, isOutput=False)
    IOTA = nc.declare_dram_parameter("IOTA", [128, 1], f32, isOutput=False)
    IDXB = nc.declare_dram_parameter("IDXB", [2 * NWIN, 128, WQ // 16], i16,
                                     isOutput=False)
    SIDXA = nc.declare_dram_parameter("SIDXA", [2, 128, SPILL // 16], i16,
                                      isOutput=False)
    SIDXB = nc.declare_dram_parameter("SIDXB", [2, 128, SPILL // 16], i16,
                                      isOutput=False)
    POS = nc.declare_dram_parameter("POS", [NWIN, 4096], bf16, isOutput=False)
    ONES = nc.declare_dram_parameter("ONES", [1, 128], bf16, isOutput=False)
    OUT = nc.declare_dram_parameter("OUT", [96, NBANK * 512], bf16, isOutput=True)

    rec_shard = nc.dram_tensor("rec_shard", [NP, 64], f32)
    rec_h1 = nc.dram_tensor("rec_h1", [NCORES * H1N, 64], f32, addr_space="Shared")
    rec_h2 = nc.dram_tensor("rec_h2", [NCORES * H2N, 64], f32, addr_space="Shared")

    with TileContext(nc) as tc:
        with (
            tc.tile_pool(name="const", bufs=1) as constp,
            tc.tile_pool(name="htp", bufs=2) as htp,
            tc.tile_pool(name="h1p", bufs=1) as h1p,
            tc.tile_pool(name="recp", bufs=3) as recp,
            tc.tile_pool(name="posp", bufs=2) as posp,
            tc.tile_pool(name="Mp", bufs=2) as Mp,
            tc.tile_pool(name="gBp", bufs=2) as gBp,
            tc.tile_pool(name="bPkp", bufs=3) as bPkp,
            tc.tile_pool(name="cBp", bufs=3) as cBp,
            tc.tile_pool(name="prodp", bufs=3) as prodp,
            tc.tile_pool(name="outp", bufs=3) as outp,
            tc.tile_pool(name="psT", bufs=2, space="PSUM") as psT,
            tc.tile_pool(name="ps2", bufs=2, space="PSUM") as ps2,
            tc.tile_pool(name="psA", bufs=2, space="PSUM") as psA,
            tc.tile_pool(name="psL", bufs=2, space="PSUM") as psL,
        ):
            w1_s = constp.tile([C, 2, HID], f32)
            nc.sync.dma_start(w1_s[:], W1[:].rearrange("t c h -> c t h"))
            b1_s = constp.tile([HID, 2, 1], f32)
            nc.sync.dma_start(b1_s[:], B1[:].rearrange("t h o -> h t o"))
            w2_s = constp.tile([HID, 2, R], bf16)
            nc.gpsimd.dma_start(w2_s[:], W2[:].rearrange("t h r -> h t r"))
            b2_s = constp.tile([128, 2, R], f32)
            nc.sync.dma_start(b2_s[:], B2[:].rearrange("t p r -> p t r"))
            gbd_s = constp.tile([128, 96], bf16)
            nc.sync.dma_start(gbd_s[:], GBD[:])
            iota_s = constp.tile([128, 1], f32)
            nc.sync.dma_start(iota_s[:], IOTA[:])
            idxb_all = constp.tile([128, 2 * NWIN, WQ // 16], i16)
            nc.sync.dma_start(idxb_all[:], IDXB[:].rearrange("t p x -> p t x"))
            sidxa_s = constp.tile([128, 2, SPILL // 16], i16)
            nc.sync.dma_start(sidxa_s[:], SIDXA[:].rearrange("t p x -> p t x"))
            sidxb_s = constp.tile([128, 2, SPILL // 16], i16)
            nc.sync.dma_start(sidxb_s[:], SIDXB[:].rearrange("t p x -> p t x"))
            ident = constp.tile([128, 128], f32)
            make_identity(nc, ident[:])
            ones_s = constp.tile([1, 128], bf16)
            nc.sync.dma_start(ones_s[:], ONES[:])
            atab = constp.tile([128, NWIN, 32], bf16)

            # ---- MLP passes; each pass ends with its half-shard AllGather
            cc_insts = []
            rec_dmas = []
            for (p0, psz) in ((0, H1N), (H1N, H2N)):
                h1t = {}
                for t in range(2):
                    for b in range(B):
                        h1x = h1p.tile([HID, max(H1N, H2N)], bf16, tag=f"h1_{t}_{b}")
                        h1t[(t, b)] = h1x
                for n0 in range(0, psz, 512):
                    csz = min(512, psz - n0)
                    htc = htp.tile([C, B, 512], f32, tag="ht")
                    nc.sync.dma_start(
                        htc[:, :, :csz],
                        HT[:, :, p0 + n0:p0 + n0 + csz].rearrange("b c n -> c b n"),
                    )
                    for t in range(2):
                        for b in range(B):
                            p1 = psT.tile([HID, 512], f32, tag="px")
                            nc.tensor.matmul(
                                p1[:, :csz],
                                w1_s[:, t, :],
                                htc[:, b, :csz],
                            )
                            nc.scalar.activation(
                                h1t[(t, b)][:, n0:n0 + csz], p1[:, :csz],
                                mybir.ActivationFunctionType.Relu,
                                bias=b1_s[:, t, :], scale=1.0,
                            )
                for s in range(psz // 128):
                    rec = recp.tile([128, 64], f32, tag="rec")
                    for t in range(2):
                        for b in range(B):
                            p2 = ps2.tile([128, R], f32, tag="p2")
                            nc.tensor.matmul(
                                p2[:],
                                h1t[(t, b)][:, s * 128:(s + 1) * 128],
                                w2_s[:, t, :],
                            )
                            co = 32 * t + 16 * b
                            nc.vector.tensor_add(
                                rec[:, co:co + 16], p2[:], b2_s[:, t, :]
                            )
                    n0 = p0 + s * 128
                    nc.scalar.copy(atab[:, n0 // 128, :], rec[:, 0:32])
                    di = nc.sync.dma_start(rec_shard[n0:n0 + 128, :], rec[:])
                    rec_dmas.append(di)
                dst = rec_h1 if p0 == 0 else rec_h2
                cc = nc.gpsimd.collective_compute(
                    "AllGather",
                    mybir.AluOpType.bypass,
                    replica_groups=[list(range(NCORES))],
                    ins=[rec_shard[p0:p0 + psz, :]],
                    outs=[dst[:]],
                )
                for di in rec_dmas:
                    add_dep_helper(cc.ins, di.ins, True, "cc waits rec dmas")
                if cc_insts:
                    add_dep_helper(cc.ins, cc_insts[-1].ins, True, "cc order")
                cc_insts.append(cc)

            # ---- window phase
            qn = 0
            for w in range(NWIN):
                posr = posp.tile([1, 4096], bf16, tag="pos")
                nc.sync.dma_start(posr[:], POS[w:w + 1, :])
                Mt = Mp.tile([128, 4096], bf16, tag="M")
                for pc in range(8):
                    pb = psT.tile([128, 512], f32, tag="px")
                    nc.tensor.matmul(
                        pb[:], ones_s[:], posr[:, 512 * pc:512 * (pc + 1)],
                        start=True, stop=True,
                    )
                    nc.vector.tensor_scalar(
                        Mt[:, 512 * pc:512 * (pc + 1)], pb[:], iota_s[:], None,
                        mybir.AluOpType.is_equal,
                    )
                for h in range(2):
                    rec_src = rec_h1 if h == 0 else rec_h2
                    gB = gBp.tile([128, WQ // 128, 64], f32, tag="gB")
                    gb_i = nc.gpsimd.dma_gather(
                        gB[:], rec_src[:], idxb_all[:, 2 * w + h, :],
                        num_idxs=WQ, num_idxs_reg=WQ, elem_size=64,
                        single_packet=False, queue_num=qn % 4,
                    )
                    qn += 1
                    add_dep_helper(gb_i.ins, cc_insts[h].ins, True, "gather waits cc")

                    # expansion: aI bank [128, 512], 4 sub-matmuls
                    aIb = psA.tile([128, 512], f32, tag="aI")
                    for si in range(4):
                        nc.tensor.matmul(
                            aIb[32 * si:32 * (si + 1), :],
                            atab[:, w, :],
                            Mt[:, 2048 * h + 512 * si:2048 * h + 512 * (si + 1)],
                            start=True, stop=True,
                            tile_position=(0, 32 * si),
                        )

                    bPk = bPkp.tile([128, WQ // 128, 32], f32, tag="bPk")
                    nc.scalar.copy(bPk[:], gB[:, :, 32:64])
                    pTB = psT.tile([128, 512], f32, tag="px")
                    for gg in range(4):
                        nc.tensor.transpose(
                            pTB[:, 128 * gg:128 * (gg + 1)],
                            bPk[:, 4 * gg:4 * (gg + 1), :],
                            ident[:],
                        )
                    cB = cBp.tile([128, 512], f32, tag="cBf")
                    nc.scalar.copy(cB[:], pTB[:])

                    prod = prodp.tile([128, 512], bf16, tag="prod")
                    nc.vector.tensor_mul(prod[:], aIb[:], cB[:])

                    pL = psL.tile([96, 512], f32, tag="pL")
                    nc.tensor.matmul(pL[:], gbd_s[:], prod[:], start=True, stop=True)
                    outS = outp.tile([96, 512], bf16, tag="outS")
                    nc.scalar.copy(outS[:], pL[:])
                    bank = 2 * w + h
                    nc.sync.dma_start(OUT[:, 512 * bank:512 * (bank + 1)], outS[:])

            # ---- spill phase: one v2-style TG per J-half
            for h in range(2):
                rec_src = rec_h1 if h == 0 else rec_h2
                gA = gBp.tile([128, SPILL // 128, 64], f32, tag="sgA")
                ga_i = nc.gpsimd.dma_gather(
                    gA[:], rec_shard[:], sidxa_s[:, h, :],
                    num_idxs=SPILL, num_idxs_reg=SPILL, elem_size=64,
                    single_packet=False, queue_num=qn % 4,
                )
                qn += 1
                for di in rec_dmas:
                    add_dep_helper(ga_i.ins, di.ins, True, "spillA waits rec")
                gB2 = gBp.tile([128, SPILL // 128, 64], f32, tag="sgB")
                gb_i = nc.gpsimd.dma_gather(
                    gB2[:], rec_src[:], sidxb_s[:, h, :],
                    num_idxs=SPILL, num_idxs_reg=SPILL, elem_size=64,
                    single_packet=False, queue_num=qn % 4,
                )
                qn += 1
                add_dep_helper(gb_i.ins, cc_insts[h].ins, True, "spillB waits cc")

                prodS = prodp.tile([128, SPILL // 128, 32], f32, tag="sprod")
                nc.vector.tensor_mul(prodS[:], gA[:, :, 0:32], gB2[:, :, 32:64])
                for jj in range(2):
                    pT = psT.tile([128, 512], f32, tag="px")
                    for gg in range(4):
                        j = 4 * jj + gg
                        nc.tensor.transpose(
                            pT[:, 128 * gg:128 * (gg + 1)],
                            prodS[:, 4 * j:4 * (j + 1), :],
                            ident[:],
                        )
                    cS = cBp.tile([128, 512], bf16, tag="cB")
                    nc.scalar.copy(cS[:], pT[:])
                    pL = psL.tile([96, 512], f32, tag="pL")
                    nc.tensor.matmul(pL[:], gbd_s[:], cS[:], start=True, stop=True)
                    outS = outp.tile([96, 512], bf16, tag="outS")
                    nc.scalar.copy(outS[:], pL[:])
                    bank = 2 * NWIN + 2 * h + jj
                    nc.sync.dma_start(OUT[:, 512 * bank:512 * (bank + 1)], outS[:])

    nc.finalize()
    return nc


# ---------------------------------------------------------------- host
def _wrap_idx(flat_idx, kg):
    """[kg] int16 -> [128, kg//16] wrapped-16, replicated x8."""
    w = flat_idx.reshape(kg // 16, 16).T
    return np.tile(w, (8, 1))


def kernel(X, edge_index, W1s, b1s, W2s, b2s, W1d, b1d, W2d, b2d, gamma):
    X = np.asarray(X)
    edge_index = np.asarray(edge_index)
    H = np.ascontiguousarray(X[:, -1]).astype(np.float32)          # (B, N, C)
    Hp = np.zeros((B, NPAD, C), np.float32)
    Hp[:, :N] = H

    I = edge_index[0].astype(np.int64)
    J = edge_index[1].astype(np.int64)

    # J-side rows in the AllGather'd half tables (per-rank interleaved)
    rJ = J // NP
    iJ = J % NP
    in1 = iJ < H1N
    rowJ = np.where(in1, H1N * rJ + iJ, H2N * rJ + (iJ - H1N))
    hJ = np.where(in1, 0, 1)

    coreof = I // NP
    Iloc = I - coreof * NP
    wof = Iloc // 128
    posof = Iloc % 128

    if () not in _PROGRAM_CACHE:
        _PROGRAM_CACHE[()] = build_program()
    nc = _PROGRAM_CACHE[()]

    W1 = np.stack([W1s, W1d]).astype(np.float32)
    B1 = np.stack([b1s, b1d]).astype(np.float32)[:, :, None]
    W2 = np.stack([W2s, W2d]).astype(np.float32)
    B2 = np.stack(
        [np.tile(b2s[None, :], (128, 1)), np.tile(b2d[None, :], (128, 1))]
    ).astype(np.float32)

    gbd = np.zeros((128, 96), np.float32)
    gT = np.asarray(gamma, np.float32).T
    for g in range(4):
        for b in range(B):
            gbd[32 * g + 16 * b:32 * g + 16 * b + 16,
                24 * g + 12 * b:24 * g + 12 * b + 12] = gT
    GBD = gbd.astype(BF16)
    IOTA = np.arange(128, dtype=np.float32)[:, None]
    ONES_h = np.ones((1, 128), BF16)

    # record r -> posrep col within a half's 2048-col block
    r2 = np.arange(WQ)
    colmap = 512 * ((r2 // 128) % 4) + 128 * (r2 // 512) + r2 % 128

    in_maps = []
    unperm = []
    for c in range(NCORES):
        sel = np.nonzero(coreof == c)[0]
        wc, hc = wof[sel], hJ[sel]
        key = 2 * wc + hc
        order = np.argsort(key, kind="stable")
        sel_s = sel[order]
        key_s = key[order]
        cnts = np.bincount(key_s, minlength=2 * NWIN)
        starts = np.zeros(2 * NWIN + 1, np.int64)
        starts[1:] = np.cumsum(cnts)

        posrep = np.full((NWIN, 4096), 255.0, np.float32)
        idxB = np.zeros((2 * NWIN, WQ), np.int16)
        pad_pos = np.full(EPAD, -1, np.int64)
        spills = [[], []]
        for w in range(NWIN):
            for h in range(2):
                k = 2 * w + h
                ed = sel_s[starts[k]:starts[k + 1]]
                if len(ed) > WQ:
                    spills[h].append(ed[WQ:])
                    ed = ed[:WQ]
                ncnt = len(ed)
                posrep[w, 2048 * h + colmap[:ncnt]] = posof[ed]
                idxB[k, :ncnt] = rowJ[ed].astype(np.int16)
                pad_pos[2048 * k:2048 * k + ncnt] = ed

        sidxA = np.zeros((2, SPILL), np.int16)
        sidxB = np.zeros((2, SPILL), np.int16)
        for h in range(2):
            sl = (np.concatenate(spills[h]) if spills[h]
                  else np.empty(0, np.int64))
            assert len(sl) <= SPILL, f"core {c} half {h} spill {len(sl)}"
            scnt = len(sl)
            sidxA[h, :scnt] = Iloc[sl].astype(np.int16)
            sidxB[h, :scnt] = rowJ[sl].astype(np.int16)
            base = 2048 * (2 * NWIN + 2 * h)
            pad_pos[base:base + scnt] = sl
        unperm.append(pad_pos)

        IDXB_w = np.zeros((2 * NWIN, 128, WQ // 16), np.int16)
        for k in range(2 * NWIN):
            IDXB_w[k] = _wrap_idx(idxB[k], WQ)
        SIDXA_w = np.stack([_wrap_idx(sidxA[h], SPILL) for h in range(2)])
        SIDXB_w = np.stack([_wrap_idx(sidxB[h], SPILL) for h in range(2)])
        POS_w = posrep.astype(BF16)

        HTs = np.ascontiguousarray(
            Hp[:, c * NP:(c + 1) * NP, :].transpose(0, 2, 1)
        )
        in_maps.append({
            "HT": HTs, "W1": W1, "B1": B1, "W2": W2, "B2": B2,
            "GBD": GBD, "IOTA": IOTA, "IDXB": IDXB_w,
            "SIDXA": SIDXA_w, "SIDXB": SIDXB_w, "POS": POS_w,
            "ONES": ONES_h,
        })

    import os
    import tempfile
    trace = bool(os.environ.get("BASS_KERNEL_TRACE"))
    tdir = None
    if trace:
        base = "/root/problem/work"
        tdir = tempfile.mkdtemp(prefix="ktrace_", dir=base if os.path.isdir(base) else None)
    res = run_bass_kernel_spmd(
        nc, in_maps, list(range(NCORES)), trace=trace, tmpdir=tdir,
    )
    if trace:
        kernel.last_trace_dir = tdir
        kernel.last_exec_time_ns = res.exec_time_ns

    logits = np.empty((B, L, E), np.float32)
    for c in range(NCORES):
        dev = res.results[c]["OUT"].astype(np.float32)             # (96, NBANK*512)
        # partition p = 24*k + 12*b + l ; col = 512*Tb + 128*g + e ;
        # padded pos = 2048*Tb + 512*g + 128*k + e
        dv = dev.reshape(4, 2, L, NBANK, 4, 128)                   # (k, b, l, Tb, g, e)
        dv = dv.transpose(1, 2, 3, 4, 0, 5).reshape(B, L, EPAD)
        pad_pos = unperm[c]
        valid = pad_pos >= 0
        logits[:, :, pad_pos[valid]] = dv[:, :, valid]
    return logits
